# revision 1
# baseline (speedup 1.0000x reference)
"""ContactMapHead bilinear pair-scoring kernel for 8 trn2 NeuronCores.

Math: for each batch b, logits[b, p] = h[b, i_p] @ W @ h[b, j_p] + bias,
where (i_p, j_p) enumerate position pairs (upper triangle, k=1, when the
masks keep every position — the general case is handled too).

This equals S_b = (h_b @ W) @ h_b^T + bias followed by a pair gather.
S_b is a 512x512 matrix per batch; total device work = two 512^3 matmuls
per batch (memory-bound at this size).

Sharding (8 cores): core c computes rows [r0, r0+128) of S_b for batch
b = c // 4, r0 = (c % 4) * 128.  W and hs_b^T are replicated per core;
each core also gets its own pre-transposed row-slice hs_rows^T.  The
host assembles S (2, 512, 512) from the 8 row-blocks and gathers the
pair indices (pure unshard/reindex).

Device program (per core), P = 128 partitions, all fp32, raw bass
(manual semaphores, no Tile entry/exit barriers).  All DRAM inputs are
host-swizzled to partition-major (128, X) so every DMA descriptor is a
large contiguous run:
    w    (128, 2048): w[p, kc*512 + :] = W[kc*128 + p, :]
    hst  (128, 2048): hst[p, hc*512 + j] = hs[b, j, hc*128 + p]
    hsrt (128, 512):  hsrt[p, kc*128 + m] = hs[b, r0 + m, kc*128 + p]
    bias (1,)
    out  (128, 512):  S rows r0..r0+127 (+bias)

  stage 1 (PE): GT[hc] (128h x 128m) += lhsT=W[kc, hc-cols] x rhs=hsrt[kc]
  copy  (DVE): gt_sb[:, hc] <- GT[hc]
  stage 2 (PE): ps (128m x 512j) += lhsT=gt_sb[:, hc] x rhs=hst[hc]
  epilogue (DVE+DMA): out = ps + bias, in two column halves on two rings
"""

import numpy as np

_B, _L, _H = 2, 512, 512
_P = 128
_KC = _H // _P          # 4 contraction chunks
_GROUPS = 4             # row-blocks per batch
_RB = _L // _GROUPS     # 128 rows per core
_NCORES = 8

# Dev/profiling knobs (used by test.py only; harness leaves them alone).
TRACE = False
TRACE_KWARGS = {}
LAST_RESULTS = None

_STATE = {}


def _build_nc():
    """Build (once) the raw-bass module for one core's row-block."""
    if "nc" in _STATE:
        return _STATE["nc"]

    from concourse import bacc, mybir

    f32 = mybir.dt.float32
    nc = bacc.Bacc("TRN2", target_bir_lowering=False, debug=False)

    w_d = nc.dram_tensor("w", [_P, 2048], f32, kind="ExternalInput")
    hst_d = nc.dram_tensor("hst", [_P, 2048], f32, kind="ExternalInput")
    hsrt_d = nc.dram_tensor("hsrt", [_P, 512], f32, kind="ExternalInput")
    bias_d = nc.dram_tensor("bias", [1], f32, kind="ExternalInput")
    out_d = nc.dram_tensor("out", [_RB, _L], f32, kind="ExternalOutput")

    w_sb = nc.alloc_sbuf_tensor("w_sb", [_P, 2048], f32)
    hst_sb = nc.alloc_sbuf_tensor("hst_sb", [_P, 2048], f32)
    hsrt_sb = nc.alloc_sbuf_tensor("hsrt_sb", [_P, 512], f32)
    bias_sb = nc.alloc_sbuf_tensor("bias_sb", [_P, 1], f32)
    gt_sb = nc.alloc_sbuf_tensor("gt_sb", [_P, 512], f32)
    out_sb = nc.alloc_sbuf_tensor("out_sb", [_P, _L], f32)
    warm_sb = nc.alloc_sbuf_tensor("warm_sb", [_P, 512], f32)
    pgt = [nc.alloc_psum_tensor(f"pgt{h}", [_P, _P], f32) for h in range(_KC)]
    ps = nc.alloc_psum_tensor("ps", [_P, _L], f32)
    pwarm = nc.alloc_psum_tensor("pwarm", [_P, _L], f32)

    s_w = [nc.alloc_semaphore(f"s_w{k}") for k in range(_KC)]  # +16 each
    s_hr = nc.alloc_semaphore("s_hr")      # +16 hsrt
    s_hst = [nc.alloc_semaphore(f"s_hst{h}") for h in range(_KC)]  # +16 each
    s_bias = nc.alloc_semaphore("s_bias")  # +16 bias
    s_gt_pe = nc.alloc_semaphore("s_gt_pe")  # +1 per stage-1 group
    s_gt_v = nc.alloc_semaphore("s_gt_v")    # +1 per gt copy
    s_s = nc.alloc_semaphore("s_s")        # +1 stage-2 done
    s_out = nc.alloc_semaphore("s_out")    # +1 per epilogue half
    s_od = nc.alloc_semaphore("s_od")      # +16 per out-DMA half
    s_warm = nc.alloc_semaphore("s_warm")  # +1 warmup scratch zeroed

    _Q = _L // 4

    with nc.Block(no_gpsimd_drain=True) as block:

        @block.sync
        def _(sync):
            # critical inputs for stage 1 first
            sync.dma_start(out=hsrt_sb[:], in_=hsrt_d[:]).then_inc(s_hr, 16)
            sync.dma_start(out=w_sb[:, 0:512], in_=w_d[:, 0:512]).then_inc(
                s_w[0], 16
            )
            sync.dma_start(out=w_sb[:, 512:1024], in_=w_d[:, 512:1024]).then_inc(
                s_w[1], 16
            )
            sync.dma_start(
                out=bias_sb[:], in_=bias_d[:].to_broadcast((_P, 1))
            ).then_inc(s_bias, 16)
            for q in (0, 1):
                sync.wait_ge(s_out, q * 2 + 1)
                sync.dma_start(
                    out=out_d[:, q * 256 : q * 256 + _Q],
                    in_=out_sb[:, q * 256 : q * 256 + _Q],
                ).then_inc(s_od, 16)
            sync.wait_ge(s_od, 64)

        @block.scalar
        def _(scalar):
            scalar.dma_start(out=w_sb[:, 1024:1536], in_=w_d[:, 1024:1536]).then_inc(
                s_w[2], 16
            )
            scalar.dma_start(out=w_sb[:, 1536:2048], in_=w_d[:, 1536:2048]).then_inc(
                s_w[3], 16
            )
            for hc in range(_KC):
                scalar.dma_start(
                    out=hst_sb[:, hc * 512 : (hc + 1) * 512],
                    in_=hst_d[:, hc * 512 : (hc + 1) * 512],
                ).then_inc(s_hst[hc], 16)
            for q in (0, 1):
                scalar.wait_ge(s_out, q * 2 + 2)
                scalar.dma_start(
                    out=out_d[:, q * 256 + _Q : (q + 1) * 256],
                    in_=out_sb[:, q * 256 + _Q : (q + 1) * 256],
                ).then_inc(s_od, 16)
            scalar.wait_ge(s_od, 64)

        @block.tensor
        def _(tensor):
            # HAM warmup: keep the PE array busy on zeros so the clock gate
            # opens (1.2 -> 2.4 GHz) before the real matmuls arrive.
            # Span the input-DMA window (~5us): inputs all land together at
            # ~13us because SDMA round-robins the rings, so the PE must stay
            # busy until then or HAM re-throttles to 1.2 GHz.
            tensor.wait_ge(s_warm, 1)
            for _ in range(4):
                nc.tensor.matmul(
                    pwarm[:],
                    lhsT=warm_sb[:, 0:_P],
                    rhs=warm_sb[:],
                    start=True,
                    stop=True,
                )
            # kc-outer so round kc only needs W chunk kc (chases the DMAs)
            tensor.wait_ge(s_hr, 16)
            for kc in range(_KC):
                tensor.wait_ge(s_w[kc], 16)
                for hc in range(_KC):
                    mm = nc.tensor.matmul(
                        pgt[hc][:],
                        lhsT=w_sb[:, kc * 512 + hc * _P : kc * 512 + (hc + 1) * _P],
                        rhs=hsrt_sb[:, kc * _P : (kc + 1) * _P],
                        start=(kc == 0),
                        stop=(kc == _KC - 1),
                    )
                    if kc == _KC - 1:
                        mm.then_inc(s_gt_pe, 1)
            for hc in range(_KC):
                tensor.wait_ge(s_gt_v, hc + 1)
                tensor.wait_ge(s_hst[hc], 16)
                mm = nc.tensor.matmul(
                    ps[:],
                    lhsT=gt_sb[:, hc * _P : (hc + 1) * _P],
                    rhs=hst_sb[:, hc * 512 : (hc + 1) * 512],
                    start=(hc == 0),
                    stop=(hc == _KC - 1),
                )
            mm.then_inc(s_s, 1)

        @block.vector
        def _(vector):
            nc.vector.memset(warm_sb[:], 0.0).then_inc(s_warm, 1)
            for hc in range(_KC):
                vector.wait_ge(s_gt_pe, hc + 1)
                nc.vector.tensor_copy(
                    gt_sb[:, hc * _P : (hc + 1) * _P], pgt[hc][:]
                ).then_inc(s_gt_v, 1)
            vector.wait_ge(s_s, 1)
            vector.wait_ge(s_bias, 16)
            # quarters, alternating sync/scalar consumers so the two out-DMA
            # rings start as early as possible and receipts pipeline
            for q in range(4):
                nc.vector.tensor_scalar_add(
                    out_sb[:, q * _Q : (q + 1) * _Q],
                    ps[:, q * _Q : (q + 1) * _Q],
                    bias_sb[:, 0:1],
                ).then_inc(s_out, 1)

    nc.compile()
    _STATE["nc"] = nc
    return nc


def _swizzle(a):
    """(512, X) row-major -> (128, 4*X): partition p gets rows p, 128+p, ..."""
    x = a.shape[1]
    return np.ascontiguousarray(
        a.reshape(_KC, _P, x).transpose(1, 0, 2).reshape(_P, _KC * x)
    )


def _device_scores(hs, w, bias):
    """Compute S[b, i, j] = (hs_b @ W @ hs_b^T)[i, j] + bias on 8 cores."""
    global LAST_RESULTS
    from concourse.bass_utils import run_bass_kernel_spmd

    nc = _build_nc()

    w_p = _swizzle(w)
    hst = [np.ascontiguousarray(hs[b].T) for b in range(_B)]
    hst_p = [_swizzle(h) for h in hst]
    in_maps = []
    for c in range(_NCORES):
        b, rc = divmod(c, _GROUPS)
        r0 = rc * _RB
        in_maps.append(
            {
                "w": w_p,
                "hst": hst_p[b],
                "hsrt": _swizzle(hst[b][:, r0 : r0 + _RB]),
                "bias": bias,
            }
        )

    kwargs = dict(TRACE_KWARGS) if TRACE else {}
    res = run_bass_kernel_spmd(
        nc, in_maps, core_ids=list(range(_NCORES)), trace=TRACE, **kwargs
    )
    LAST_RESULTS = res

    s = np.empty((_B, _L, _L), np.float32)
    for c in range(_NCORES):
        b, rc = divmod(c, _GROUPS)
        s[b, rc * _RB : (rc + 1) * _RB, :] = res.results[c]["out"]
    return s


def kernel(hidden_states, W, b, attention_mask, special_tokens_mask):
    hs = np.ascontiguousarray(np.asarray(hidden_states, dtype=np.float32))
    w = np.ascontiguousarray(np.asarray(W, dtype=np.float32)[0])
    bias = np.asarray(b, dtype=np.float32).reshape(1)
    am = np.asarray(attention_mask)
    sm = np.asarray(special_tokens_mask)

    # Pair indices from the (constant) masks — mirrors the reference.
    aa_mask = (am[0] == 1) & (sm[0] == 0)
    aa_positions = np.nonzero(aa_mask)[0]
    n_aa = aa_positions.shape[0]
    if n_aa < 2:
        return np.zeros((hs.shape[0], 0), dtype=np.float32)
    tri_i, tri_j = np.triu_indices(n_aa, k=1)
    idx_i = aa_positions[tri_i]
    idx_j = aa_positions[tri_j]

    if hs.shape != (_B, _L, _H) or w.shape != (_H, _H):
        # Defensive fallback for unexpected shapes (never hit by the spec).
        g = hs @ w
        s = np.einsum("bik,bjk->bij", g, hs) + bias[0]
        return s[:, idx_i, idx_j].astype(np.float32)

    s = _device_scores(hs, w, bias)  # bias already added on device
    return s[:, idx_i, idx_j].astype(np.float32)



# revision 8
# speedup vs baseline: 1.3436x; 1.3436x over previous
"""ContactMapHead bilinear pair-scoring kernel for 8 trn2 NeuronCores.

Math: for each batch b, logits[b, p] = h[b, i_p] @ W @ h[b, j_p] + bias,
where (i_p, j_p) enumerate position pairs (upper triangle, k=1, when the
masks keep every position — the general case is handled too).

This equals S_b = (h_b @ W) @ h_b^T followed by a pair gather (+bias,
added on host: 0.05% of the FLOPs).  S_b is a 512x512 matrix per batch;
total device work = two 512^3 matmuls per batch (memory-bound).

Sharding (8 cores): core c computes rows [r0, r0+128) of S_b for batch
b = c // 4, r0 = (c % 4) * 128.  All device data is bf16 (the harness
tolerance is 2e-2; bf16 end-to-end is ~4e-3), halving both HBM traffic
and PE passes vs fp32 (which runs LOW_HIGH 2-pass).

Per-core inputs, host-swizzled partition-major so every DMA line is
contiguous:
    w   (128, 2048) bf16: w[p, kc*512 + h] = W[kc*128 + p, h]
    hst (128, 2048) bf16, j-quarter-major:
        hst[p, jq*512 + hc*128 + jj] = hs[b, jq*128 + jj, hc*128 + p]
    out (128, 512) bf16: S rows r0..r0+127 (no bias)
The stage-1 rhs (own rows, transposed) is exactly hst quarter rc —
no separate tensor needed.

Device program (per core), P = 128 partitions, raw bass:
  stage 1 (PE): GT[hc] (128h x 128m) += lhsT=W[kc, hc-cols] x rhs=hst[rc-quarter, kc]
  copy  (DVE): gt_sb[:, hc] <- GT[hc]  (fp32 psum -> bf16)
  stage 2 (PE), per j-quarter jq (own quarter first, then in DMA order):
        ps[:, jq] += lhsT=gt_sb[:, hc] x rhs=hst[jq, hc]
  epilogue (DVE): out_sb[:, jq] <- ps[:, jq] (bf16), out-DMA per quarter
Input DMAs are split across the sync and scalar queues so the stage-1
dependencies (own quarter + w01) land first and stage 2 chases the
remaining quarters; out-DMA quarters alternate between the two queues.
"""

import numpy as np
import ml_dtypes

_BF16 = np.dtype(ml_dtypes.bfloat16)

_B, _L, _H = 2, 512, 512
_P = 128
_KC = _H // _P          # 4 contraction chunks
_GROUPS = 4             # row-blocks per batch
_RB = _L // _GROUPS     # 128 rows per core
_NCORES = 8
_NWARM = 16             # HAM warmup matmuls (128-wide bf16)

# Dev/profiling knobs (used by test.py only; harness leaves them alone).
TRACE = False
TRACE_KWARGS = {}
LAST_RESULTS = None

_STATE = {}


def _build_nc():
    """Build (once) the raw-bass module shared by all 8 cores.

    SPMD runs ONE program on all cores, so nothing core-specific is baked
    in: the host rotates each core's hst quarters so slot 0 is always the
    core's own row-block (stage-1 rhs), and un-rotates the output columns.
    In module coordinates rc == 0.
    """
    if "nc" in _STATE:
        return _STATE["nc"]
    rc = 0

    from concourse import bacc, mybir

    f32 = mybir.dt.float32
    bf16 = mybir.dt.bfloat16
    nc = bacc.Bacc("TRN2", target_bir_lowering=False, debug=False)

    w_d = nc.dram_tensor("w", [_P, 2048], bf16, kind="ExternalInput")
    hst_d = nc.dram_tensor("hst", [_P, 2048], bf16, kind="ExternalInput")
    out_d = nc.dram_tensor("out", [_RB, _L], bf16, kind="ExternalOutput")

    w_sb = nc.alloc_sbuf_tensor("w_sb", [_P, 2048], bf16)
    hst_sb = nc.alloc_sbuf_tensor("hst_sb", [_P, 2048], bf16)
    gt_sb = nc.alloc_sbuf_tensor("gt_sb", [_P, 512], bf16)
    out_sb = nc.alloc_sbuf_tensor("out_sb", [_P, _L], bf16)
    warm_sb = nc.alloc_sbuf_tensor("warm_sb", [_P, _P], bf16)
    pgt = [nc.alloc_psum_tensor(f"pgt{h}", [_P, _P], f32) for h in range(_KC)]
    # one PSUM tensor per stage-2 quarter: the epilogue reads quarter q
    # while quarter q+1's accumulation group is still open, which is only
    # legal across distinct tensors.  Warmup reuses psq[0] (strictly
    # earlier in PE program order).
    psq = [nc.alloc_psum_tensor(f"psq{q}", [_P, _P], f32) for q in range(4)]
    pwarm = psq[0]

    s_w01 = nc.alloc_semaphore("s_w01")    # +16 w chunks 0-1
    s_w23 = nc.alloc_semaphore("s_w23")    # +16 w chunks 2-3
    s_hq = [nc.alloc_semaphore(f"s_hq{q}") for q in range(4)]  # +16 each
    s_warm = nc.alloc_semaphore("s_warm")  # +1 warmup scratch zeroed
    s_gt_pe = nc.alloc_semaphore("s_gt_pe")  # +1 per stage-1 hc group
    s_gt_v = nc.alloc_semaphore("s_gt_v")    # +1 per gt copy
    s_s = nc.alloc_semaphore("s_s")        # +1 per stage-2 quarter
    s_out = nc.alloc_semaphore("s_out")    # +1 per epilogue quarter
    s_od = nc.alloc_semaphore("s_od")      # +16 per out-DMA quarter

    # stage-2 quarter order: own quarter first, then DMA arrival order
    others = [q for q in range(4) if q != rc]
    order = [rc] + others

    def hq(q):
        return hst_sb[:, q * 512 : (q + 1) * 512], hst_d[:, q * 512 : (q + 1) * 512]

    with nc.Block(no_gpsimd_drain=True) as block:

        @block.sync
        def _(sync):
            o, i = hq(rc)
            sync.dma_start(out=o, in_=i).then_inc(s_hq[rc], 16)
            sync.dma_start(out=w_sb[:, 1024:2048], in_=w_d[:, 1024:2048]).then_inc(
                s_w23, 16
            )
            o, i = hq(others[2])
            sync.dma_start(out=o, in_=i).then_inc(s_hq[others[2]], 16)
            for idx in (0, 2):
                jq = order[idx]
                sync.wait_ge(s_out, idx + 1)
                sync.dma_start(
                    out=out_d[:, jq * _P : (jq + 1) * _P],
                    in_=out_sb[:, jq * _P : (jq + 1) * _P],
                ).then_inc(s_od, 16)
            sync.wait_ge(s_od, 64)

        @block.scalar
        def _(scalar):
            scalar.dma_start(out=w_sb[:, 0:1024], in_=w_d[:, 0:1024]).then_inc(
                s_w01, 16
            )
            for q in others[:2]:
                o, i = hq(q)
                scalar.dma_start(out=o, in_=i).then_inc(s_hq[q], 16)
            for idx in (1, 3):
                jq = order[idx]
                scalar.wait_ge(s_out, idx + 1)
                scalar.dma_start(
                    out=out_d[:, jq * _P : (jq + 1) * _P],
                    in_=out_sb[:, jq * _P : (jq + 1) * _P],
                ).then_inc(s_od, 16)
            scalar.wait_ge(s_od, 64)

        @block.tensor
        def _(tensor):
            # HAM warmup: keep the PE array busy on zeros so the clock gate
            # opens (1.2 -> 2.4 GHz) while the input DMAs are in flight.
            tensor.wait_ge(s_warm, 1)
            for _ in range(_NWARM):
                nc.tensor.matmul(
                    pwarm[:],
                    lhsT=warm_sb[:],
                    rhs=warm_sb[:],
                    start=True,
                    stop=True,
                )
            # stage 1, kc-outer so round kc only needs its W half
            tensor.wait_ge(s_hq[rc], 16)
            for kc in range(_KC):
                if kc == 0:
                    tensor.wait_ge(s_w01, 16)
                elif kc == 2:
                    tensor.wait_ge(s_w23, 16)
                for hc in range(_KC):
                    mm = nc.tensor.matmul(
                        pgt[hc][:],
                        lhsT=w_sb[:, kc * 512 + hc * _P : kc * 512 + (hc + 1) * _P],
                        rhs=hst_sb[:, rc * 512 + kc * _P : rc * 512 + (kc + 1) * _P],
                        start=(kc == 0),
                        stop=(kc == _KC - 1),
                    )
                    if kc == _KC - 1:
                        mm.then_inc(s_gt_pe, 1)
            # stage 2, per j-quarter, chasing the hst DMAs
            tensor.wait_ge(s_gt_v, 4)
            for idx, jq in enumerate(order):
                if jq != rc:
                    tensor.wait_ge(s_hq[jq], 16)
                for hc in range(_KC):
                    mm = nc.tensor.matmul(
                        psq[idx][:],
                        lhsT=gt_sb[:, hc * _P : (hc + 1) * _P],
                        rhs=hst_sb[:, jq * 512 + hc * _P : jq * 512 + (hc + 1) * _P],
                        start=(hc == 0),
                        stop=(hc == _KC - 1),
                    )
                    if hc == _KC - 1:
                        mm.then_inc(s_s, 1)

        @block.vector
        def _(vector):
            nc.vector.memset(warm_sb[:], 0.0).then_inc(s_warm, 1)
            for hc in range(_KC):
                vector.wait_ge(s_gt_pe, hc + 1)
                nc.vector.tensor_copy(
                    gt_sb[:, hc * _P : (hc + 1) * _P], pgt[hc][:]
                ).then_inc(s_gt_v, 1)
            for idx, jq in enumerate(order):
                vector.wait_ge(s_s, idx + 1)
                nc.vector.tensor_copy(
                    out_sb[:, jq * _P : (jq + 1) * _P],
                    psq[idx][:],
                ).then_inc(s_out, 1)

    nc.compile()
    _STATE["nc"] = nc
    return nc


def _swizzle_w(w):
    """(512, 512) -> (128, 2048) bf16: w_p[p, kc*512+h] = W[kc*128+p, h]."""
    return np.ascontiguousarray(
        w.reshape(_KC, _P, _H).transpose(1, 0, 2).reshape(_P, _KC * _H)
    ).astype(_BF16)


def _swizzle_hst(hs_b):
    """(512, 512) -> (128, 2048) bf16, j-quarter-major:
    hst[p, jq*512 + hc*128 + jj] = hs_b[jq*128+jj, hc*128+p]."""
    return np.ascontiguousarray(
        hs_b.reshape(4, _P, _KC, _P).transpose(3, 0, 2, 1).reshape(_P, 2048)
    ).astype(_BF16)


def _device_scores(hs, w):
    """Compute S[b, i, j] = (hs_b @ W @ hs_b^T)[i, j] on 8 cores (no bias)."""
    global LAST_RESULTS
    from concourse.bass_utils import run_bass_kernel_spmd

    nc = _build_nc()

    w_p = _swizzle_w(w)
    hst_p = [_swizzle_hst(np.ascontiguousarray(hs[b])) for b in range(_B)]
    in_maps = []
    for c in range(_NCORES):
        b, rc = divmod(c, _GROUPS)
        # rotate quarters so the core's own quarter sits at slot 0 and the
        # compiled (rc=0) program reads its own rows from slot 0
        perm = [rc] + [q for q in range(4) if q != rc]
        h = hst_p[b].reshape(_P, 4, 512)[:, perm, :].reshape(_P, 2048)
        in_maps.append({"w": w_p, "hst": np.ascontiguousarray(h)})

    kwargs = dict(TRACE_KWARGS) if TRACE else {}
    res = run_bass_kernel_spmd(
        nc, in_maps, core_ids=list(range(_NCORES)), trace=TRACE, **kwargs
    )
    LAST_RESULTS = res

    s = np.empty((_B, _L, _L), np.float32)
    for c in range(_NCORES):
        b, rc = divmod(c, _GROUPS)
        out = np.asarray(res.results[c]["out"]).astype(np.float32)
        # compiled program wrote columns in permuted quarter space: quarter
        # slot q holds j-range perm[q]; undo the permutation
        perm = [rc] + [q for q in range(4) if q != rc]
        o = np.empty_like(out)
        for slot, jq in enumerate(perm):
            o[:, jq * _P : (jq + 1) * _P] = out[:, slot * _P : (slot + 1) * _P]
        s[b, rc * _RB : (rc + 1) * _RB, :] = o
    return s


def kernel(hidden_states, W, b, attention_mask, special_tokens_mask):
    hs = np.ascontiguousarray(np.asarray(hidden_states, dtype=np.float32))
    w = np.ascontiguousarray(np.asarray(W, dtype=np.float32)[0])
    bias = np.asarray(b, dtype=np.float32).reshape(1)
    am = np.asarray(attention_mask)
    sm = np.asarray(special_tokens_mask)

    # Pair indices from the (constant) masks — mirrors the reference.
    aa_mask = (am[0] == 1) & (sm[0] == 0)
    aa_positions = np.nonzero(aa_mask)[0]
    n_aa = aa_positions.shape[0]
    if n_aa < 2:
        return np.zeros((hs.shape[0], 0), dtype=np.float32)
    tri_i, tri_j = np.triu_indices(n_aa, k=1)
    idx_i = aa_positions[tri_i]
    idx_j = aa_positions[tri_j]

    if hs.shape != (_B, _L, _H) or w.shape != (_H, _H):
        # Defensive fallback for unexpected shapes (never hit by the spec).
        g = hs @ w
        s = np.einsum("bik,bjk->bij", g, hs) + bias[0]
        return s[:, idx_i, idx_j].astype(np.float32)

    s = _device_scores(hs, w)
    return (s[:, idx_i, idx_j] + bias[0]).astype(np.float32)


# revision 12
# speedup vs baseline: 1.3539x; 1.0077x over previous
"""ContactMapHead bilinear pair-scoring kernel for 8 trn2 NeuronCores.

Math: for each batch b, logits[b, p] = h[b, i_p] @ W @ h[b, j_p] + bias,
where (i_p, j_p) enumerate position pairs (upper triangle, k=1, when the
masks keep every position — the general case is handled too).

This equals S_b = (h_b @ W) @ h_b^T followed by a pair gather (+bias,
added on host: 0.05% of the FLOPs).  S_b is a 512x512 matrix per batch;
total device work = two 512^3 matmuls per batch (memory-bound).

Sharding (8 cores): core c computes rows [r0, r0+128) of S_b for batch
b = c // 4, r0 = (c % 4) * 128.  All device data is bf16 (the harness
tolerance is 2e-2; bf16 end-to-end is ~4e-3), halving both HBM traffic
and PE passes vs fp32 (which runs LOW_HIGH 2-pass).

Per-core inputs, host-swizzled partition-major so every DMA line is
contiguous:
    w   (128, 2048) bf16: w[p, kc*512 + h] = W[kc*128 + p, h]
    hst (128, 2048) bf16, j-quarter-major:
        hst[p, jq*512 + hc*128 + jj] = hs[b, jq*128 + jj, hc*128 + p]
    out (128, 512) bf16: S rows r0..r0+127 (no bias)
The stage-1 rhs (own rows, transposed) is exactly hst quarter rc —
no separate tensor needed.

Device program (per core), P = 128 partitions, raw bass:
  stage 1 (PE): GT[hc] (128h x 128m) += lhsT=W[kc, hc-cols] x rhs=hst[rc-quarter, kc]
  copy  (DVE): gt_sb[:, hc] <- GT[hc]  (fp32 psum -> bf16)
  stage 2 (PE), per j-quarter jq (own quarter first, then in DMA order):
        ps[:, jq] += lhsT=gt_sb[:, hc] x rhs=hst[jq, hc]
  epilogue (DVE): out_sb[:, jq] <- ps[:, jq] (bf16), out-DMA per quarter
Input DMAs are split across the sync and scalar queues so the stage-1
dependencies (own quarter + w01) land first and stage 2 chases the
remaining quarters; out-DMA quarters alternate between the two queues.
"""

import numpy as np
import ml_dtypes

_BF16 = np.dtype(ml_dtypes.bfloat16)

_B, _L, _H = 2, 512, 512
_P = 128
_KC = _H // _P          # 4 contraction chunks
_GROUPS = 4             # row-blocks per batch
_RB = _L // _GROUPS     # 128 rows per core
_NCORES = 8
_NWARM = 16             # HAM warmup matmuls (128-wide bf16)

# Dev/profiling knobs (used by test.py only; harness leaves them alone).
TRACE = False
TRACE_KWARGS = {}
LAST_RESULTS = None

_STATE = {}


def _build_nc():
    """Build (once) the raw-bass module shared by all 8 cores.

    SPMD runs ONE program on all cores, so nothing core-specific is baked
    in: the host rotates each core's hst quarters so slot 0 is always the
    core's own row-block (stage-1 rhs), and un-rotates the output columns.
    In module coordinates rc == 0.
    """
    if "nc" in _STATE:
        return _STATE["nc"]
    rc = 0

    from concourse import bacc, mybir

    f32 = mybir.dt.float32
    bf16 = mybir.dt.bfloat16
    nc = bacc.Bacc("TRN2", target_bir_lowering=False, debug=False)

    w_d = nc.dram_tensor("w", [_P, 2048], bf16, kind="ExternalInput")
    hst_d = nc.dram_tensor("hst", [_P, 2048], bf16, kind="ExternalInput")
    out_d = nc.dram_tensor("out", [_RB, _L], bf16, kind="ExternalOutput")

    w_sb = nc.alloc_sbuf_tensor("w_sb", [_P, 2048], bf16)
    hst_sb = nc.alloc_sbuf_tensor("hst_sb", [_P, 2048], bf16)
    gt_sb = nc.alloc_sbuf_tensor("gt_sb", [_P, 512], bf16)
    out_sb = nc.alloc_sbuf_tensor("out_sb", [_P, _L], bf16)
    warm_sb = nc.alloc_sbuf_tensor("warm_sb", [_P, _P], bf16)
    pgt = [nc.alloc_psum_tensor(f"pgt{h}", [_P, _P], f32) for h in range(_KC)]
    # one PSUM tensor per stage-2 quarter: the epilogue reads quarter q
    # while quarter q+1's accumulation group is still open, which is only
    # legal across distinct tensors.  Warmup reuses psq[0] (strictly
    # earlier in PE program order).
    psq = [nc.alloc_psum_tensor(f"psq{q}", [_P, _P], f32) for q in range(4)]
    pwarm = psq[0]

    s_w01 = nc.alloc_semaphore("s_w01")    # +16 w chunks 0-1
    s_w23 = nc.alloc_semaphore("s_w23")    # +16 w chunks 2-3
    s_h01 = nc.alloc_semaphore("s_h01")    # +16 hst slots 0-1
    s_h23 = nc.alloc_semaphore("s_h23")    # +16 hst slots 2-3
    s_warm = nc.alloc_semaphore("s_warm")  # +1 warmup scratch zeroed
    s_gt_pe = nc.alloc_semaphore("s_gt_pe")  # +1 per stage-1 hc group
    s_gt_v = nc.alloc_semaphore("s_gt_v")    # +1 per gt copy
    s_s = nc.alloc_semaphore("s_s")        # +1 per stage-2 quarter
    s_out = nc.alloc_semaphore("s_out")    # +1 per epilogue quarter
    s_od = nc.alloc_semaphore("s_od")      # +16 per out-DMA quarter

    # stage-2 processes hst slots 0..3 in order; the host rotates each
    # core's quarters so slot 0 is the core's own row-block
    order = list(range(4))

    with nc.Block(no_gpsimd_drain=True) as block:

        @block.sync
        def _(sync):
            # hst slots 0-1 first (slot 0 is the stage-1 rhs), then W 2-3.
            # 1024-col halves keep every DMA line at 2KB: 1KB lines halve
            # the effective per-engine HBM rate.
            sync.dma_start(out=hst_sb[:, 0:1024], in_=hst_d[:, 0:1024]).then_inc(
                s_h01, 16
            )
            sync.dma_start(out=w_sb[:, 1024:2048], in_=w_d[:, 1024:2048]).then_inc(
                s_w23, 16
            )
            for idx in (0, 2):
                jq = order[idx]
                sync.wait_ge(s_out, idx + 1)
                sync.dma_start(
                    out=out_d[:, jq * _P : (jq + 1) * _P],
                    in_=out_sb[:, jq * _P : (jq + 1) * _P],
                ).then_inc(s_od, 16)
            sync.wait_ge(s_od, 64)

        @block.scalar
        def _(scalar):
            scalar.dma_start(out=w_sb[:, 0:1024], in_=w_d[:, 0:1024]).then_inc(
                s_w01, 16
            )
            scalar.dma_start(out=hst_sb[:, 1024:2048], in_=hst_d[:, 1024:2048]).then_inc(
                s_h23, 16
            )
            for idx in (1, 3):
                jq = order[idx]
                scalar.wait_ge(s_out, idx + 1)
                scalar.dma_start(
                    out=out_d[:, jq * _P : (jq + 1) * _P],
                    in_=out_sb[:, jq * _P : (jq + 1) * _P],
                ).then_inc(s_od, 16)
            scalar.wait_ge(s_od, 64)

        @block.tensor
        def _(tensor):
            # HAM warmup: keep the PE array busy on zeros so the clock gate
            # opens (1.2 -> 2.4 GHz) while the input DMAs are in flight.
            tensor.wait_ge(s_warm, 1)
            for _ in range(_NWARM):
                nc.tensor.matmul(
                    pwarm[:],
                    lhsT=warm_sb[:],
                    rhs=warm_sb[:],
                    start=True,
                    stop=True,
                )
            # stage 1, kc-outer so round kc only needs its W half
            tensor.wait_ge(s_h01, 16)
            for kc in range(_KC):
                if kc == 0:
                    tensor.wait_ge(s_w01, 16)
                elif kc == 2:
                    tensor.wait_ge(s_w23, 16)
                for hc in range(_KC):
                    mm = nc.tensor.matmul(
                        pgt[hc][:],
                        lhsT=w_sb[:, kc * 512 + hc * _P : kc * 512 + (hc + 1) * _P],
                        rhs=hst_sb[:, rc * 512 + kc * _P : rc * 512 + (kc + 1) * _P],
                        start=(kc == 0),
                        stop=(kc == _KC - 1),
                    )
                    if kc == _KC - 1:
                        mm.then_inc(s_gt_pe, 1)
            # stage 2, per j-quarter, chasing the hst DMAs; slot 0 starts
            # as each gt chunk's cast lands (overlaps the cast chain)
            for idx, jq in enumerate(order):
                if idx == 2:
                    tensor.wait_ge(s_h23, 16)
                for hc in range(_KC):
                    if idx == 0:
                        tensor.wait_ge(s_gt_v, hc + 1)
                    mm = nc.tensor.matmul(
                        psq[idx][:],
                        lhsT=gt_sb[:, hc * _P : (hc + 1) * _P],
                        rhs=hst_sb[:, jq * 512 + hc * _P : jq * 512 + (hc + 1) * _P],
                        start=(hc == 0),
                        stop=(hc == _KC - 1),
                    )
                    if hc == _KC - 1:
                        mm.then_inc(s_s, 1)

        @block.vector
        def _(vector):
            nc.vector.memset(warm_sb[:], 0.0).then_inc(s_warm, 1)
            for hc in range(_KC):
                vector.wait_ge(s_gt_pe, hc + 1)
                nc.vector.tensor_copy(
                    gt_sb[:, hc * _P : (hc + 1) * _P], pgt[hc][:]
                ).then_inc(s_gt_v, 1)
            for idx, jq in enumerate(order):
                vector.wait_ge(s_s, idx + 1)
                nc.vector.tensor_copy(
                    out_sb[:, jq * _P : (jq + 1) * _P],
                    psq[idx][:],
                ).then_inc(s_out, 1)

    nc.compile()
    _STATE["nc"] = nc
    return nc


def _swizzle_w(w):
    """(512, 512) -> (128, 2048) bf16: w_p[p, kc*512+h] = W[kc*128+p, h]."""
    return np.ascontiguousarray(
        w.reshape(_KC, _P, _H).transpose(1, 0, 2).reshape(_P, _KC * _H)
    ).astype(_BF16)


def _swizzle_hst(hs_b):
    """(512, 512) -> (128, 2048) bf16, j-quarter-major:
    hst[p, jq*512 + hc*128 + jj] = hs_b[jq*128+jj, hc*128+p]."""
    return np.ascontiguousarray(
        hs_b.reshape(4, _P, _KC, _P).transpose(3, 0, 2, 1).reshape(_P, 2048)
    ).astype(_BF16)


def _device_scores(hs, w):
    """Compute S[b, i, j] = (hs_b @ W @ hs_b^T)[i, j] on 8 cores (no bias)."""
    global LAST_RESULTS
    from concourse.bass_utils import run_bass_kernel_spmd

    nc = _build_nc()

    w_p = _swizzle_w(w)
    hst_p = [_swizzle_hst(np.ascontiguousarray(hs[b])) for b in range(_B)]
    in_maps = []
    for c in range(_NCORES):
        b, rc = divmod(c, _GROUPS)
        # rotate quarters so the core's own quarter sits at slot 0 and the
        # compiled (rc=0) program reads its own rows from slot 0
        perm = [rc] + [q for q in range(4) if q != rc]
        h = hst_p[b].reshape(_P, 4, 512)[:, perm, :].reshape(_P, 2048)
        in_maps.append({"w": w_p, "hst": np.ascontiguousarray(h)})

    kwargs = dict(TRACE_KWARGS) if TRACE else {}
    res = run_bass_kernel_spmd(
        nc, in_maps, core_ids=list(range(_NCORES)), trace=TRACE, **kwargs
    )
    LAST_RESULTS = res

    s = np.empty((_B, _L, _L), np.float32)
    for c in range(_NCORES):
        b, rc = divmod(c, _GROUPS)
        out = np.asarray(res.results[c]["out"]).astype(np.float32)
        # compiled program wrote columns in permuted quarter space: quarter
        # slot q holds j-range perm[q]; undo the permutation
        perm = [rc] + [q for q in range(4) if q != rc]
        o = np.empty_like(out)
        for slot, jq in enumerate(perm):
            o[:, jq * _P : (jq + 1) * _P] = out[:, slot * _P : (slot + 1) * _P]
        s[b, rc * _RB : (rc + 1) * _RB, :] = o
    return s


def kernel(hidden_states, W, b, attention_mask, special_tokens_mask):
    hs = np.ascontiguousarray(np.asarray(hidden_states, dtype=np.float32))
    w = np.ascontiguousarray(np.asarray(W, dtype=np.float32)[0])
    bias = np.asarray(b, dtype=np.float32).reshape(1)
    am = np.asarray(attention_mask)
    sm = np.asarray(special_tokens_mask)

    # Pair indices from the (constant) masks — mirrors the reference.
    aa_mask = (am[0] == 1) & (sm[0] == 0)
    aa_positions = np.nonzero(aa_mask)[0]
    n_aa = aa_positions.shape[0]
    if n_aa < 2:
        return np.zeros((hs.shape[0], 0), dtype=np.float32)
    tri_i, tri_j = np.triu_indices(n_aa, k=1)
    idx_i = aa_positions[tri_i]
    idx_j = aa_positions[tri_j]

    if hs.shape != (_B, _L, _H) or w.shape != (_H, _H):
        # Defensive fallback for unexpected shapes (never hit by the spec).
        g = hs @ w
        s = np.einsum("bik,bjk->bij", g, hs) + bias[0]
        return s[:, idx_i, idx_j].astype(np.float32)

    s = _device_scores(hs, w)
    return (s[:, idx_i, idx_j] + bias[0]).astype(np.float32)
